# revision 39
# baseline (speedup 1.0000x reference)
"""Trainium2 Bass kernel for nn_MultiHeadAttention_3539053052118.

GQA attention (B=2, S=2048, HID=2048, 16 q-heads, 4 kv-heads, RoPE, causal)
distributed over 8 NeuronCores: 2-way data-parallel over batch x 4-way
tensor-parallel over kv-head groups. Each core computes q/kv projections for
its 4 q-heads + 1 kv-head (bf16 matmuls; f32 inputs are loaded as bf16 by
DMA-ing the high halfword of each f32 word), RoPE, causal flash attention
with a globally software-pipelined scores->exp->sums/ctx chain; each head's
context is AllGather-ed (bf16) within the 4-core batch group as soon as it
is ready, and the o_proj accumulates per-wave into SBUF so the collectives
overlap attention. Each core produces a distinct 512-column slice of the
output. The host only shards/aliases/bit-views inputs and concatenates
slices.
"""

import math
import sys
import types

sys.path.insert(0, "/opt/trn_rl_repo")

import antenv  # noqa: F401

if "antenv.axon_hooks" not in sys.modules:
    _hooks = types.ModuleType("antenv.axon_hooks")
    _hook_box = {"hook": None}
    _hooks.set_axon_ntff_profile_hook = lambda h: _hook_box.__setitem__("hook", h)
    _hooks.get_axon_ntff_profile_hook = lambda: _hook_box["hook"]
    sys.modules["antenv.axon_hooks"] = _hooks
    try:
        from trn_agent_boot.trn_boot import _ntff_profile_via_ctypes

        _hooks.set_axon_ntff_profile_hook(
            _ntff_profile_via_ctypes("/opt/axon/libaxon_pjrt.so")
        )
    except Exception:
        pass

import numpy as np
import ml_dtypes
import concourse.bass as bass
import concourse.mybir as mybir
import concourse.tile as tile
from concourse import bacc
from concourse import bass_utils
from concourse.masks import make_identity

F32 = mybir.dt.float32
F32R = mybir.dt.float32r
BF16 = mybir.dt.bfloat16
I32 = mybir.dt.int32
AF = mybir.ActivationFunctionType
ALU = mybir.AluOpType

B, S, HID = 2, 2048, 2048
NH, NKV = 16, 4
HD = 128
ROPE_BASE = 10000.0
PI = math.pi

N_CORES = 8
TP = 4
HG = NH // TP  # 4 q heads per core
GROUPS = [[0, 1, 2, 3], [4, 5, 6, 7]]

NKC = HID // 128  # 16 contraction tiles
NQC = S // 512  # 4 q/n chunks
NST = S // 128  # 16 s tiles
OC = 512  # output columns per core

_CACHE = {}


def _build():
    nc = bacc.Bacc("TRN2", target_bir_lowering=False, debug=False, num_devices=N_CORES)

    xT = nc.dram_tensor("xT", [HID, S], F32, kind="ExternalInput").ap()
    wqT = nc.dram_tensor("wqT", [HID, HG * HD], F32, kind="ExternalInput").ap()
    wkT = nc.dram_tensor("wkT", [HID, HD], F32, kind="ExternalInput").ap()
    wvT = nc.dram_tensor("wvT", [HID, HD], F32, kind="ExternalInput").ap()
    woT = nc.dram_tensor("woT", [HID, OC], F32, kind="ExternalInput").ap()
    pos = nc.dram_tensor("pos", [1, S], I32, kind="ExternalInput").ap()
    out = nc.dram_tensor("out_slice", [S, OC], F32, kind="ExternalOutput").ap()

    # per-head collective bounce buffers (separate tensors so AG(h) only
    # depends on head h's writes)
    cc_in = [
        [nc.dram_tensor(f"cc_in{h}_{hf}", [HD, S // 2], BF16).ap() for hf in range(2)]
        for h in range(HG)
    ]
    cc_out = [
        [
            nc.dram_tensor(f"cc_out{h}_{hf}", [TP * HD, S // 2], BF16).ap()
            for hf in range(2)
        ]
        for h in range(HG)
    ]
    # quarter-granularity buffers for the very last gathers (tail latency)
    cc_in_q = [nc.dram_tensor(f"cc_inq{j}", [HD, 512], BF16).ap() for j in range(2)]
    cc_out_q = [
        nc.dram_tensor(f"cc_outq{j}", [TP * HD, 512], BF16).ap() for j in range(2)
    ]
    # tiny warm-up collective: absorbs first-collective setup cost and
    # re-syncs the cores right at kernel start
    cc_wout = nc.dram_tensor("cc_wout", [TP, 64], BF16).ap()

    # ---- inline constants ----
    half = HD // 2
    invf = 1.0 / (ROPE_BASE ** (np.arange(half) / half))
    invf_t = np.tile(invf, 2)[:, None].astype(np.float32) / (2 * PI)  # turns
    invf_c = nc.inline_tensor(invf_t, "invf").ap()
    R = np.zeros((HD, HD), np.float32)
    for p in range(half):
        R[p, p + half] = -1.0
    for p in range(half, HD):
        R[p, p - half] = 1.0
    permRT_c = nc.inline_tensor(
        np.ascontiguousarray(R.T).astype(ml_dtypes.bfloat16), "permRT"
    ).ap()
    ones_row_c = nc.inline_tensor(np.ones((1, 128), np.float32), "ones_row").ap()
    ones_row_bf_c = nc.inline_tensor(
        np.ones((1, 128), ml_dtypes.bfloat16), "ones_row_bf"
    ).ap()
    ones_col_c = nc.inline_tensor(
        np.ones((128, 1), ml_dtypes.bfloat16), "ones_col"
    ).ap()
    # causal bias: B[p, j] = -30000 where key p > query j (within diag subtile)
    btri = np.where(
        np.arange(128)[:, None] > np.arange(128)[None, :], -30000.0, 0.0
    ).astype(np.float32)
    btriT_c = nc.inline_tensor(
        np.ascontiguousarray(btri.T).astype(ml_dtypes.bfloat16), "btriT"
    ).ap()
    ident_bf_c = nc.inline_tensor(
        np.eye(128, dtype=ml_dtypes.bfloat16), "ident_bf"
    ).ap()
    warm_c = nc.inline_tensor(np.ones((1, 64), ml_dtypes.bfloat16), "warm").ap()

    with tile.TileContext(nc) as tc:
        with (
            tc.tile_pool(name="const", bufs=1) as cpool,
            tc.tile_pool(name="w", bufs=1) as wpool,
            tc.tile_pool(name="attn", bufs=2) as apool,
        ):
            qkvpool = tc.alloc_tile_pool(name="qkv", bufs=1)
            # ---- constants ----
            invf_sb = cpool.tile([HD, 1], F32)
            nc.sync.dma_start(out=invf_sb[:, :], in_=invf_c[:, :])
            permRT_sb = cpool.tile([HD, HD], BF16)
            nc.sync.dma_start(out=permRT_sb[:, :], in_=permRT_c[:, :])
            ones_row_f32r = cpool.tile([1, 128], F32R)
            nc.sync.dma_start(
                out=ones_row_f32r[:, :], in_=ones_row_c.bitcast(F32R)[:, :]
            )
            ones_col_sb = cpool.tile([128, 1], BF16)
            nc.sync.dma_start(out=ones_col_sb[:, :], in_=ones_col_c[:, :])
            btriT_sb = cpool.tile([128, 128], BF16)
            nc.sync.dma_start(out=btriT_sb[:, :], in_=btriT_c[:, :])
            ident_bf = cpool.tile([128, 128], BF16)
            nc.sync.dma_start(out=ident_bf[:, :], in_=ident_bf_c[:, :])
            ident_sb = cpool.tile([128, 128], F32)
            make_identity(nc, ident_sb[:, :])

            # warm-up AllGather: first in the CC queue, runs during phase 0/1
            nc.gpsimd.collective_compute(
                "AllGather",
                mybir.AluOpType.bypass,
                replica_groups=GROUPS,
                ins=[warm_c[:, :]],
                outs=[cc_wout[:, :]],
            )

            # rope tables are built lazily (interleaved into chunk-0 of
            # phase 1) so their serial DMA->DVE->PE->ACT chain doesn't
            # delay the projection start
            tabtmp = tc.alloc_tile_pool(name="tabtmp", bufs=1)
            sinT = cpool.tile([128, S], BF16, tag="tab_sin", name="tab_sin")
            cosT = cpool.tile([128, S], BF16, tag="tab_cos", name="tab_cos")
            pos_fs = []
            for q in range(NQC):
                pos_i = tabtmp.tile([1, 512], I32, tag=f"pos_i{q}", name=f"pos_i{q}")
                nc.sync.dma_start(out=pos_i[:, :], in_=pos[:, q * 512 : (q + 1) * 512])
                pos_f = tabtmp.tile([1, 512], F32R, tag=f"pos_f{q}", name=f"pos_f{q}")
                nc.vector.tensor_copy(pos_f[:, :], pos_i[:, :])
                pos_fs.append(pos_f)

            # ---- weights: persistent bf16 ----
            wk_sb = [
                wpool.tile([128, HD], BF16, tag=f"wk{i}", name=f"wk{i}")
                for i in range(NKC)
            ]
            wv_sb = [
                wpool.tile([128, HD], BF16, tag=f"wv{i}", name=f"wv{i}")
                for i in range(NKC)
            ]
            wo_sb = [
                wpool.tile([128, OC], BF16, tag=f"wo{i}", name=f"wo{i}")
                for i in range(NKC)
            ]
            # persistent qkv storage (bf16)
            q_sb = [
                qkvpool.tile([128, S], BF16, tag=f"q{h}", name=f"q{h}")
                for h in range(HG)
            ]
            k_sb = qkvpool.tile([128, S], BF16, tag="k", name="k_sb")
            vT_sb = qkvpool.tile([128, S], F32, tag="vT", name="vT_sb")
            v_sb = [
                qkvpool.tile([128, HD], BF16, tag=f"v{i}", name=f"v{i}")
                for i in range(NST)
            ]

            wqpool = tc.alloc_tile_pool(name="wq", bufs=1)
            wq_sb = [
                wqpool.tile([128, HG * HD], BF16, tag=f"wq{i}", name=f"wq{i}")
                for i in range(NKC)
            ]
            wtpool = tc.alloc_tile_pool(name="wt", bufs=2)
            xspool = tc.alloc_tile_pool(name="xs", bufs=1)
            xbpool = tc.alloc_tile_pool(name="xb", bufs=1)
            psP = tc.alloc_tile_pool(name="psP", bufs=1, space="PSUM")
            psR = tc.alloc_tile_pool(name="psR", bufs=1, space="PSUM")

            # interleave chunk-0 x with weights so kt=0 operands land first;
            # x casts go to the scalar engine (idle in phase 1), weight casts
            # to DVE (idle at startup) so neither serializes the other
            x_pending = {}
            for kt in range(NKC):
                sl = slice(kt * 128, (kt + 1) * 128)
                t = xspool.tile([128, 512], F32, tag=f"x{kt}", name=f"x_0_{kt}")
                nc.sync.dma_start(out=t[:, :], in_=xT[sl, 0:512])
                tb = xbpool.tile([128, 512], BF16, tag=f"xb{kt}", name=f"xb_0_{kt}")
                nc.scalar.activation(tb[:, :], t[:, :], AF.Copy)
                x_pending[kt] = tb
                wqt = wtpool.tile([128, HG * HD], F32, tag="wqt", name=f"wqt{kt}")
                nc.sync.dma_start(out=wqt[:, :], in_=wqT[sl, :])
                nc.vector.tensor_copy(wq_sb[kt][:, :], wqt[:, :])
                wkt = wtpool.tile([128, 2 * HD], F32, tag="wkt", name=f"wkt{kt}")
                nc.sync.dma_start(out=wkt[:, 0:HD], in_=wkT[sl, :])
                nc.sync.dma_start(out=wkt[:, HD : 2 * HD], in_=wvT[sl, :])
                nc.vector.tensor_copy(wk_sb[kt][:, :], wkt[:, 0:HD])
                nc.vector.tensor_copy(wv_sb[kt][:, :], wkt[:, HD : 2 * HD])

            # ---- phase 1: projections + rope + v transpose ----
            # rope/v-transpose of chunk q is deferred and interleaved into the
            # PE stream of chunk q+1 (or early attention) so the PE never
            # waits head-of-line on the DVE rope chain.
            pending_items = []  # closures emitting one deferred PE item each
            psO_box = [None]  # filled once the attention-phase psO pool exists

            def emit_rope(qq, idx, pool=None, tag="rot"):
                ns_ = slice(qq * 512, (qq + 1) * 512)
                tgt = q_sb[idx][:, ns_] if idx < HG else k_sb[:, ns_]
                ps_rot = (pool or psR).tile(
                    [128, 512], F32, tag=tag, name=f"rot{qq}_{idx}"
                )
                nc.tensor.matmul(
                    ps_rot[:, :], permRT_sb[:, :], tgt, start=True, stop=True
                )
                tmp = apool.tile([128, 512], BF16, tag="ropetmp", name=f"rt{qq}_{idx}")
                nc.vector.tensor_tensor(tmp[:, :], tgt, cosT[:, ns_], op=ALU.mult)
                nc.vector.tensor_tensor(tgt, ps_rot[:, :], sinT[:, ns_], op=ALU.mult)
                nc.vector.tensor_tensor(tgt, tgt, tmp[:, :], op=ALU.add)

            def emit_vt(stile):
                ps_v = psR.tile([128, 128], F32, tag="vt", name=f"vt{stile}")
                nc.tensor.transpose(
                    ps_v[:, :],
                    vT_sb[:, stile * 128 : (stile + 1) * 128],
                    ident_sb[:, :],
                )
                nc.vector.tensor_copy(v_sb[stile][:, :], ps_v[:, :])

            def emit_tab(q_):
                ns_ = slice(q_ * 512, (q_ + 1) * 512)
                ps_pos = psR.tile([128, 512], F32, tag="rot", name=f"ps_pos{q_}")
                nc.tensor.matmul(
                    ps_pos[:, :], ones_row_f32r[:, :], pos_fs[q_][:, :],
                    start=True, stop=True,
                )
                for add_quarter, tab in ((False, sinT), (True, cosT)):
                    nm = f"{q_}_{int(add_quarter)}"
                    t_t = tabtmp.tile([128, 512], F32, tag="t_t", name=f"t_{nm}")
                    if add_quarter:
                        nc.vector.tensor_scalar(
                            t_t[:, :], ps_pos[:, :], invf_sb[:, :], 0.25,
                            op0=ALU.mult, op1=ALU.add,
                        )
                    else:
                        nc.vector.tensor_scalar_mul(
                            t_t[:, :], ps_pos[:, :], invf_sb[:, :]
                        )
                    t_i = tabtmp.tile([128, 512], I32, tag="t_i", name=f"ti_{nm}")
                    nc.vector.tensor_copy(t_i[:, :], t_t[:, :])
                    t_f = tabtmp.tile([128, 512], F32, tag="t_f", name=f"tf_{nm}")
                    nc.vector.tensor_copy(t_f[:, :], t_i[:, :])
                    nc.vector.tensor_sub(t_t[:, :], t_t[:, :], t_f[:, :])
                    nc.scalar.activation(tab[:, ns_], t_t[:, :], AF.Sin, scale=2 * PI)

            def drain_one():
                if pending_items:
                    pending_items.pop(0)()

            for q_ in range(NQC):
                pending_items.append(lambda qq=q_: emit_tab(qq))

            for q in range(NQC):
                ns = slice(q * 512, (q + 1) * 512)
                x_bf = []
                for kt in range(NKC):
                    if q == 0:
                        tb = x_pending[kt]
                    else:
                        t = xspool.tile(
                            [128, 512], F32, tag=f"x{kt}", name=f"x_{q}_{kt}"
                        )
                        nc.sync.dma_start(
                            out=t[:, :], in_=xT[kt * 128 : (kt + 1) * 128, ns]
                        )
                        tb = xbpool.tile(
                            [128, 512], BF16, tag=f"xb{kt}", name=f"xb_{q}_{kt}"
                        )
                        nc.scalar.activation(tb[:, :], t[:, :], AF.Copy)
                    x_bf.append(tb)
                ps_proj = [
                    psP.tile([128, 512], F32, tag=f"proj{i}", name=f"proj{i}_{q}")
                    for i in range(HG + 2)
                ]
                for kt in range(NKC):
                    st, sp = kt == 0, kt == NKC - 1
                    for h in range(HG):
                        nc.tensor.matmul(
                            ps_proj[h][:, :],
                            wq_sb[kt][:, h * HD : (h + 1) * HD],
                            x_bf[kt][:, :],
                            start=st,
                            stop=sp,
                        )
                    nc.tensor.matmul(
                        ps_proj[HG][:, :], wk_sb[kt][:, :], x_bf[kt][:, :],
                        start=st, stop=sp,
                    )
                    nc.tensor.matmul(
                        ps_proj[HG + 1][:, :], wv_sb[kt][:, :], x_bf[kt][:, :],
                        start=st, stop=sp,
                    )
                    drain_one()

                # vT first so deferred/immediate v-transposes unblock early
                nc.vector.tensor_copy(vT_sb[:, ns], ps_proj[HG + 1][:, :])
                for h in range(HG):
                    nc.vector.tensor_copy(q_sb[h][:, ns], ps_proj[h][:, :])
                nc.vector.tensor_copy(k_sb[:, ns], ps_proj[HG][:, :])

                if q == NQC - 1:
                    # last chunk: v-transposes inline (psR dies with phase 1);
                    # rope drains into early attention via the psO "po" ring
                    for j in range(4):
                        emit_vt(q * 4 + j)
                else:
                    for j in range(4):
                        pending_items.append(
                            (lambda ss=q * 4 + j: emit_vt(ss))
                        )
                for idx in range(HG + 1):
                    if q == NQC - 1:
                        pending_items.append(
                            (lambda qq=q, ii=idx: emit_rope(
                                qq, ii, pool=psO_box[0], tag="po"
                            ))
                        )
                    else:
                        pending_items.append(
                            (lambda qq=q, ii=idx: emit_rope(qq, ii))
                        )

            psR.release()
            psP.release()
            xbpool.release()
            xspool.release()
            wtpool.release()
            wqpool.release()
            tabtmp.release()

            # o_proj weights: load + cast (overlaps attention)
            for kt in range(NKC):
                wtmp = apool.tile([128, OC], F32, tag="wotmp", name=f"wotmp{kt}")
                nc.sync.dma_start(out=wtmp[:, :], in_=woT[kt * 128 : (kt + 1) * 128, :])
                nc.vector.tensor_copy(wo_sb[kt][:, :], wtmp[:, :])

            # ---- phase 2: attention; AG(h) issued per head; o_proj waves ----
            ppool = tc.alloc_tile_pool(name="probs", bufs=6)
            ctxpool = tc.alloc_tile_pool(name="ctx", bufs=2)
            accpool = tc.alloc_tile_pool(name="acc", bufs=1)
            cblkpool = tc.alloc_tile_pool(name="cblk", bufs=1)
            ps2 = tc.alloc_tile_pool(name="ps2", bufs=1, space="PSUM")
            psO = tc.alloc_tile_pool(name="psO", bufs=2, space="PSUM")
            psO_box[0] = psO

            scale = float(HD**-0.5)
            anchors = {}

            # global software pipeline across the whole attention sweep:
            # each kt "slot" emits scores+exp; queued sums/ctx (lag 2) and
            # chunk-finalize work (DVE lag 0 / PE-norm lag 4) retire later so
            # the PE never waits head-of-line on exp or the DVE norm chain.
            slot_box = [0]
            pend = []  # entries: (slot, lag, is_pe, fn); fn emits instructions

            def pump(force=False):
                ran_pe = False
                while pend:
                    s0, lag, is_pe, fn = pend[0]
                    if not is_pe:
                        pend.pop(0)
                        fn()
                        continue
                    if ran_pe and not force:
                        break
                    if force or slot_box[0] - s0 >= lag:
                        pend.pop(0)
                        fn()
                        ran_pe = True
                        continue
                    break

            def emit_norm(hh, qq, craw, rf):
                rbc = apool.tile([128, 512], F32, tag="rbc", name=f"rbc{hh}_{qq}")
                nc.gpsimd.partition_broadcast(rbc[:, :], rf[:, :])
                csb = ctxpool.tile(
                    [128, 512], BF16, tag="ctxsb", name=f"cs{hh}_{qq}"
                )
                nc.vector.tensor_tensor(
                    csb[:, :], craw[:, :], rbc[:, :], op=ALU.mult
                )
                if hh == HG - 1 and qq >= 2:
                    jq = qq - 2
                    csb_dma = nc.sync.dma_start(
                        out=cc_in_q[jq][:, :], in_=csb[:, :]
                    )
                    anchors[(hh, qq)] = csb_dma.ins
                    nc.gpsimd.collective_compute(
                        "AllGather",
                        mybir.AluOpType.bypass,
                        replica_groups=GROUPS,
                        ins=[cc_in_q[jq][:, :]],
                        outs=[cc_out_q[jq][:, :]],
                    )
                    return
                hhf = qq // 2
                csb_dma = nc.sync.dma_start(
                    out=cc_in[hh][hhf][:, (qq % 2) * 512 : (qq % 2 + 1) * 512],
                    in_=csb[:, :],
                )
                anchors[(hh, qq)] = csb_dma.ins
                # fire the half-AG once both of its chunks are written (head
                # HG-1 sweeps q in reverse, so its half completes at q == 0)
                fire = (qq == 0) if hh == HG - 1 else (qq % 2 == 1)
                if fire:
                    nc.gpsimd.collective_compute(
                        "AllGather",
                        mybir.AluOpType.bypass,
                        replica_groups=GROUPS,
                        ins=[cc_in[hh][hhf][:, :]],
                        outs=[cc_out[hh][hhf][:, :]],
                    )

            for h in range(HG):
                qorder = (
                    range(NQC - 1, -1, -1) if h == HG - 1 else range(NQC)
                )
                for q in qorder:
                    nkt = 4 * q + 4
                    ps_sums = ps2.tile(
                        [1, 512], F32, tag="sums", name=f"sums{h}_{q}", bufs=1
                    )
                    ps_ctx = ps2.tile(
                        [128, 512], F32, tag="ctx", name=f"ctx{h}_{q}", bufs=2
                    )

                    def make_sums_ctx(hh, qq, kt_, psums, pctx, probs_t):
                        def fn():
                            c0_ = max(0, kt_ - 4 * qq) * 128
                            cs_ = slice(c0_, 512)
                            nkt_ = 4 * qq + 4
                            st_, sp_ = kt_ == 0, kt_ == nkt_ - 1
                            nc.tensor.matmul(
                                psums[:, cs_], ones_col_sb[:, :], probs_t[:, cs_],
                                start=st_, stop=sp_,
                            )
                            nc.tensor.matmul(
                                pctx[:, cs_], v_sb[kt_][:, :], probs_t[:, cs_],
                                start=st_, stop=sp_,
                            )
                        return fn

                    for kt in range(nkt):
                        o = kt - 4 * q
                        c0 = max(0, o) * 128  # first valid column in the chunk
                        cs = slice(c0, 512)
                        ps_s = ps2.tile(
                            [128, 512], F32, tag="scores", name=f"s{h}_{q}_{kt}", bufs=3
                        )
                        if o >= 0:
                            # causal bias on the 128 diagonal columns only;
                            # scores overwrite the rest (per-element
                            # has_written: start=True clears the bank)
                            nc.tensor.matmul(
                                ps_s[:, c0 : c0 + 128],
                                btriT_sb[:, :],
                                ident_bf[:, :],
                                start=True,
                                stop=False,
                            )
                        nc.tensor.matmul(
                            ps_s[:, cs],
                            k_sb[:, kt * 128 : (kt + 1) * 128],
                            q_sb[h][:, q * 512 + c0 : (q + 1) * 512],
                            start=o < 0,
                            stop=True,
                        )
                        pT = ppool.tile(
                            [128, 512], BF16, tag="probs", name=f"p{h}_{q}_{kt}"
                        )
                        nc.scalar.activation(pT[:, cs], ps_s[:, cs], AF.Exp, scale=scale)
                        drain_one()
                        pend.append(
                            (slot_box[0], 2, True,
                             make_sums_ctx(h, q, kt, ps_sums, ps_ctx, pT))
                        )
                        slot_box[0] += 1
                        pump()

                    def make_fin(hh, qq, psums, pctx):
                        def fn():
                            craw = apool.tile(
                                [128, 512], F32, tag="ctx_raw", name=f"cr{hh}_{qq}"
                            )
                            nc.vector.tensor_copy(craw[:, :], pctx[:, :])
                            rf = apool.tile(
                                [1, 512], F32, tag="recipf", name=f"rf{hh}_{qq}"
                            )
                            nc.vector.reciprocal_approx_fast(rf[:, :], psums[:, :])
                            emit_norm(hh, qq, craw, rf)
                        return fn

                    pend.append((slot_box[0], 0, False, make_fin(h, q, ps_sums, ps_ctx)))
            pump(force=True)

            # o_proj waves: wave h brings rows for global kt = 4r + h
            acc_sb = [
                accpool.tile([128, OC], F32, tag=f"acc{i}", name=f"acc{i}")
                for i in range(NST)
            ]
            def emit_half_wave(h, hf):
                anchor = anchors.get((h + 1, 2 if hf == 0 else 3))
                cblk = []
                for r in range(TP):
                    t = cblkpool.tile(
                        [128, S // 2], BF16, tag=f"cblk{r}",
                        name=f"cb{h}_{hf}_{r}", bufs=2,
                    )
                    d = nc.sync.dma_start(
                        out=t[:, :], in_=cc_out[h][hf][r * 128 : (r + 1) * 128, :]
                    )
                    if anchor is not None:
                        tile.add_dep_helper(
                            d.ins, anchor, False, "delay o_proj wave"
                        )
                    cblk.append(t)
                for j in range(NST // 2):
                    stile = hf * (NST // 2) + j
                    ps_po = psO.tile(
                        [128, OC], F32, tag="po", name=f"po{h}_{hf}_{j}"
                    )
                    for r in range(TP):
                        nc.tensor.matmul(
                            ps_po[:, :],
                            cblk[r][:, j * 128 : (j + 1) * 128],
                            wo_sb[4 * r + h][:, :],
                            start=r == 0,
                            stop=r == TP - 1,
                        )
                    if h == 0:
                        nc.vector.tensor_copy(acc_sb[stile][:, :], ps_po[:, :])
                    else:
                        nc.vector.tensor_tensor(
                            acc_sb[stile][:, :], acc_sb[stile][:, :], ps_po[:, :],
                            op=ALU.add,
                        )
                    if h == HG - 1:
                        nc.sync.dma_start(
                            out=out[stile * 128 : (stile + 1) * 128, :],
                            in_=acc_sb[stile][:, :],
                        )

            for h in range(HG - 1):
                for hf in range(2):
                    emit_half_wave(h, hf)
            # head HG-1 sweeps q in reverse, so its quarter gathers (q=3,2 ->
            # s-tiles 12-15, 8-11) land first and the half (q=1,0 -> s-tiles
            # 0-7) last: emit waves in arrival order to shorten the tail
            h = HG - 1
            for jq in (1, 0):
                cblk = []
                for r in range(TP):
                    t = cblkpool.tile(
                        [128, 512], BF16, tag=f"cblk{r}", name=f"cq{jq}_{r}", bufs=2
                    )
                    nc.sync.dma_start(
                        out=t[:, :], in_=cc_out_q[jq][r * 128 : (r + 1) * 128, :]
                    )
                    cblk.append(t)
                for j in range(4):
                    stile = 8 + jq * 4 + j
                    ps_po = psO.tile([128, OC], F32, tag="po", name=f"poq{jq}_{j}")
                    for r in range(TP):
                        nc.tensor.matmul(
                            ps_po[:, :],
                            cblk[r][:, j * 128 : (j + 1) * 128],
                            wo_sb[4 * r + h][:, :],
                            start=r == 0,
                            stop=r == TP - 1,
                        )
                    nc.vector.tensor_tensor(
                        acc_sb[stile][:, :], acc_sb[stile][:, :], ps_po[:, :],
                        op=ALU.add,
                    )
                    nc.sync.dma_start(
                        out=out[stile * 128 : (stile + 1) * 128, :],
                        in_=acc_sb[stile][:, :],
                    )
            emit_half_wave(HG - 1, 0)
            psO.release()
            ps2.release()
            cblkpool.release()
            accpool.release()
            ctxpool.release()
            ppool.release()
            qkvpool.release()

    nc.compile()
    return nc


def _get_nc():
    if "nc" not in _CACHE:
        _CACHE["nc"] = _build()
    return _CACHE["nc"]


def _shard(hidden_states, position_ids, Wq, Wkv, Wo):
    """Pure layout work: slice + transpose per core. No arithmetic."""
    x = np.asarray(hidden_states, np.float32)
    pos = np.asarray(position_ids, np.int32)
    Wq = np.asarray(Wq, np.float32)
    Wkv = np.asarray(Wkv, np.float32)
    Wo = np.asarray(Wo, np.float32)

    in_maps = []
    for c in range(N_CORES):
        b, g = c // TP, c % TP
        krows = g * 2 * HD + 2 * np.arange(HD)
        in_maps.append(
            {
                "xT": np.ascontiguousarray(x[b].T),
                "wqT": np.ascontiguousarray(Wq[g * OC : (g + 1) * OC].T),
                "wkT": np.ascontiguousarray(Wkv[krows].T),
                "wvT": np.ascontiguousarray(Wkv[krows + 1].T),
                "woT": np.ascontiguousarray(Wo[g * OC : (g + 1) * OC].T),
                "pos": np.ascontiguousarray(pos[b][None, :]),
            }
        )
    return in_maps


def run(hidden_states, position_ids, Wq, Wkv, Wo, trace=False):
    nc = _get_nc()
    in_maps = _shard(hidden_states, position_ids, Wq, Wkv, Wo)
    res = bass_utils.run_bass_kernel_spmd(
        nc, in_maps, core_ids=list(range(N_CORES)), trace=trace
    )
    out = np.empty((B, S, HID), np.float32)
    for c in range(N_CORES):
        b, g = c // TP, c % TP
        out[b][:, g * OC : (g + 1) * OC] = res.results[c]["out_slice"]
    return out, res


def kernel(hidden_states, position_ids, Wq, Wkv, Wo):
    out, _ = run(hidden_states, position_ids, Wq, Wkv, Wo, trace=False)
    return out
